# revision 28
# baseline (speedup 1.0000x reference)
"""DigitCapsules dynamic-routing kernel for 8 TRN2 NeuronCores.

Strategy: shard the input-capsule axis I=4096 across 8 cores (512 each).

Round 1 (uniform c=1/C) collapses to a direct contraction
s1 = (1/C) sum_{i,d} W[(i,d),(c,D)] x[b,(i,d)]: 64 accumulating matmuls
with K=128 (x-chunk stationary, W streaming) - no per-pair work at all.

Rounds 2-3 re-form u_hat pair-by-pair on the TensorEngine (block-diag
K=32 matmuls, 4-way row-tiling), processed in groups of 3 pairs:
  ACT  : evacuate u_hat PSUM->SBUF bf16 (one op/group), exp(b_ij)
  DVE  : tmp = u*v (2x bf16), softmax denom+recip, per-pair selector
         values (1/E folded INTO the selector matmul - no cn pass),
         y = u*e (2x bf16, e pre-expanded over D by an idle DMA queue)
  Pool : segmented reduce sum_D tmp -> agreement a, b_ij += a
  PE   : formation + selector matmuls (s-accumulation in PSUM)
Only the tiny per-round s_j partial [64,512] is all-reduced.

B=64, I=4096, C=32, D=16, d=16, 3 routing iterations.
"""

import numpy as np

import concourse.bass as bass
import concourse.mybir as mybir
from concourse import tile
from concourse.bass_utils import run_bass_kernel_spmd

B = 64
I_FULL = 4096
C = 32
D = 16
DSMALL = 16
CD = C * D  # 512
NCORES = 8
I_LOC = I_FULL // NCORES  # 512
NPAIR = I_LOC // 2  # 256
NGRP = NPAIR // 4  # 64 groups of 4 pairs (one pair per 32-row strip)
ROUNDS = 3
EPS = 1e-9

F32 = mybir.dt.float32
BF16 = mybir.dt.bfloat16


def _split_waits(nc, max_waits=1):
    """walrus in this toolchain rejects instructions carrying more than
    ~2 semaphore waits; move extras onto preceding same-engine NOPs."""
    for bb_wrap in nc.bb_map.values():
        bb = bb_wrap.bb
        newlist = []
        changed = False
        for inst in bb.instructions:
            si = inst.sync_info
            waits = list(si.on_wait) if si and si.on_wait else []
            if len(waits) > max_waits:
                extra, keep = waits[:-max_waits], waits[-max_waits:]
                k = 0
                while extra:
                    chunk, extra = extra[:max_waits], extra[max_waits:]
                    nop = mybir.InstNoOp(
                        name=f"{inst.name}-waitsplit{k}",
                        engine=inst.engine,
                        sync_info=mybir.SyncInfo(on_wait=chunk, on_update=[]),
                    )
                    nc.register_instruction(nop, overwrite=True)
                    newlist.append(nop)
                    k += 1
                inst.sync_info = mybir.SyncInfo(
                    on_wait=keep,
                    on_update=list(si.on_update) if si.on_update else [],
                )
                changed = True
            newlist.append(inst)
        if changed:
            bb.instructions = newlist
    return nc


# pair-group partition for the heavy rounds: super-groups of 6 pairs.
# One 6-bank PSUM tile per group; formation of g+1 waits on g's single
# ACT evacuation (PE has slack).  6 uh + 1 s1 + 1 s-accum = 8 banks.
GROUPS = [(6 * k, 6) for k in range(42)] + [(252, 4)]
assert sum(n for _, n in GROUPS) == NPAIR
NPG_MAX = 6


def _squash_chain(nc, ep, sp, s2, v_sb, pre_scale):
    """v = squash(pre_scale * s2), on [B, CD] fp32 tiles in (D, c) free
    order.  squash(s): s / (1+n2) / sqrt(n2+eps), n2 = sum_D s^2.
    Folding: with t = k*s (k=pre_scale), n2 = k^2 * sum(s^2);
    v = t * scl with scl = 1/((1+n2)*sqrt(n2+eps)) computed on n2."""
    sq = ep.tile([B, CD], F32, tag="sq")
    nc.vector.tensor_tensor(out=sq[:], in0=s2[:], in1=s2[:], op=mybir.AluOpType.mult)
    q = sp.tile([B, C], F32, tag="q")
    nc.vector.tensor_reduce(
        out=q[:],
        in_=sq[:].rearrange("p (d c) -> p c d", d=D),
        axis=mybir.AxisListType.X,
        op=mybir.AluOpType.add,
    )
    k2 = pre_scale * pre_scale
    n2 = sp.tile([B, C], F32, tag="n2")
    nc.vector.tensor_scalar_mul(n2[:], q[:], k2)
    n2e = sp.tile([B, C], F32, tag="n2e")
    nc.vector.tensor_scalar_add(n2e[:], n2[:], EPS)
    rt = sp.tile([B, C], F32, tag="rt")
    nc.scalar.activation(rt[:], n2e[:], mybir.ActivationFunctionType.Sqrt)
    on2 = sp.tile([B, C], F32, tag="on2")
    nc.vector.tensor_scalar_add(on2[:], n2[:], 1.0)
    den = sp.tile([B, C], F32, tag="den")
    nc.vector.tensor_tensor(
        out=den[:], in0=rt[:], in1=on2[:], op=mybir.AluOpType.mult
    )
    scl = sp.tile([B, C], F32, tag="scl")
    nc.vector.reciprocal(scl[:], den[:])
    # v = (k*s) * scl : fold k into scl first (cheap [B,C] op)
    sclk = sp.tile([B, C], F32, tag="sclk")
    nc.vector.tensor_scalar_mul(sclk[:], scl[:], pre_scale)
    nc.vector.tensor_tensor(
        out=v_sb[:].rearrange("p (d c) -> p d c", c=C),
        in0=s2[:].rearrange("p (d c) -> p d c", c=C),
        in1=sclk[:].unsqueeze(1).broadcast_to([B, D, C]),
        op=mybir.AluOpType.mult,
    )


def build_bass(ncores=NCORES):
    nc = bass.Bass(
        "TRN2", target_bir_lowering=False, debug=False, num_devices=ncores
    )
    xd_ext = nc.dram_tensor("xd", [128, NGRP * 128], BF16, kind="ExternalInput").ap()
    wt_ext = nc.dram_tensor("wt", [128, NGRP * CD], BF16, kind="ExternalInput").ap()
    xr_ext = nc.dram_tensor("xr", [128, NGRP * B], BF16, kind="ExternalInput").ap()
    sel_ext = nc.dram_tensor("sel", [128, B], BF16, kind="ExternalInput").ap()
    out_ext = nc.dram_tensor("out", [B, CD], F32, kind="ExternalOutput").ap()

    with tile.TileContext(nc) as tc:
        with (
            tc.tile_pool(name="persist", bufs=1) as pp,
            tc.tile_pool(name="work", bufs=2) as wp,
            tc.tile_pool(name="epi", bufs=1) as ep,
            tc.tile_pool(name="usb", bufs=4) as up,
            tc.tile_pool(name="small", bufs=2) as sp,
            tc.tile_pool(name="uh", bufs=2, space="PSUM") as uhp,
            tc.tile_pool(name="sacc1", bufs=1, space="PSUM") as sacc1p,
            tc.tile_pool(name="sacc", bufs=1, space="PSUM") as saccp,
            tc.tile_pool(name="dram", bufs=2, space="DRAM") as dp,
        ):
            xd = pp.tile([128, NGRP * 128], BF16)
            wt = pp.tile([128, NGRP * CD], BF16)
            xr = pp.tile([128, NGRP * B], BF16)
            sel = pp.tile([128, B], BF16)
            bij = pp.tile([128, NPAIR * C], BF16)
            vbc = pp.tile([128, CD], BF16)
            # all-reduce slot buffer: slot 0 = own partial, slots 1..7
            # filled by peers' remote DMAs (XOR-relative: slot k holds the
            # partial from core my_id^k).  Safe as a single buffer: a peer
            # cannot send round r+1 partials until it has passed round r's
            # semaphore wait plus a full round of compute (~200us).
            slotsA = pp.tile([128, NCORES * CD], F32)
            rsem = nc.alloc_semaphore("allred_r")
            lsem = nc.alloc_semaphore("allred_l")
            nc._allred_rsem = rsem
            nc._allred_lsem = lsem
            nc._allred_slots_tensor = slotsA.tensor.name

            # proxy gpsimd library: tensor_tensor + remote-DMA desc gen
            # without mid-kernel reloads
            from concourse import library_config
            nc.gpsimd.load_library(library_config.proxy)

            # stage inputs; wt in 4 chunks so round-1 matmuls can start
            # before the whole tensor has landed
            nc.sync.dma_start(xr[:], xr_ext)
            nc.sync.dma_start(sel[:], sel_ext)
            WCH = NGRP // 4
            for ch in range(4):
                nc.sync.dma_start(
                    wt[:, ch * WCH * CD : (ch + 1) * WCH * CD],
                    wt_ext[:, ch * WCH * CD : (ch + 1) * WCH * CD],
                )
            nc.sync.dma_start(xd[:], xd_ext)
            nc.vector.memset(bij[:], 0.0)
            nc.vector.memset(slotsA[:, 0:CD], 0.0)

            # ---------------- round 1: direct contraction --------------
            # s1_raw[b, cD] = sum_g  xr_g[128,(b)]^T @ wt[:, g-block]
            # (wt block g's 128 partitions are exactly (i,d) for the 8
            # capsules of that block; xr holds x in the matching layout)
            s1_ps = sacc1p.tile([B, CD], F32)
            for g in range(NGRP):
                nc.tensor.matmul(
                    s1_ps[:],
                    lhsT=xr[:, g * B : (g + 1) * B],
                    rhs=wt[:, g * CD : (g + 1) * CD],
                    start=(g == 0),
                    stop=(g == NGRP - 1),
                    skip_group_check=True,
                )

            for r in range(ROUNDS):
                if r == 0:
                    s_ps = s1_ps
                else:
                    s_ps = saccp.tile([B, CD], F32)

                    # producer: form a group on PE, evacuate to SBUF bf16 on
                    # ACT.  Runs LAG groups ahead of the consumer so PE/ACT
                    # fill buffers during collectives and round starts.
                    usb_tiles = {}

                    def form_group(gi):
                        p0, npg = GROUPS[gi]
                        u_sb = up.tile([128, NPG_MAX * CD], BF16, tag="usb")
                        # two 3-bank halves: formation of one half overlaps
                        # the ACT evacuation of the other
                        for h0 in range(0, npg, 3):
                            nh = min(3, npg - h0)
                            uh = uhp.tile([128, 3 * CD], F32, tag="uh")
                            for j in range(nh):
                                p = p0 + h0 + j
                                g, st = p // 4, p % 4
                                nc.tensor.matmul(
                                    uh[:, j * CD : (j + 1) * CD],
                                    lhsT=xd[32 * st : 32 * st + 32, g * 128 : (g + 1) * 128],
                                    rhs=wt[32 * st : 32 * st + 32, g * CD : (g + 1) * CD],
                                    start=True,
                                    stop=True,
                                    tile_position=(32 * st, 0),
                                )
                            nc.scalar.activation(
                                u_sb[:, (h0) * CD : (h0 + nh) * CD],
                                uh[:, : nh * CD],
                                mybir.ActivationFunctionType.Copy,
                            )
                        usb_tiles[gi] = u_sb

                    LAG = 2
                    for gi in range(LAG):
                        form_group(gi)
                    for gi, (p0, npg) in enumerate(GROUPS):
                        if gi + LAG < len(GROUPS):
                            form_group(gi + LAG)
                        w = npg * CD
                        u_sb = usb_tiles.pop(gi)
                        # DVE: tmp = u * v (2x bf16; v broadcast per pair
                        # via middle-dim 0-stride, innermost stays packed)
                        # u free layout is (pair, D, c): D-major so later
                        # broadcasts/reductions keep the innermost packed.
                        tmp = wp.tile([128, NPG_MAX * CD], BF16, tag="tmp")
                        nc.vector.tensor_tensor(
                            out=tmp[:, :w].rearrange("p (j q) -> p j q", q=CD),
                            in0=u_sb[:, :w].rearrange("p (j q) -> p j q", q=CD),
                            in1=vbc[:].unsqueeze(1).broadcast_to([128, npg, CD]),
                            op=mybir.AluOpType.mult,
                        )
                        # DVE: agreement a = sum_D tmp as a 4-step halving
                        # tree over the OUTER D dim (TT-adds run 2x on
                        # packed bf16; TensorReduce would be stuck at 1x).
                        # |a|~0.04 on |b|~0.1: bf16 partials cost ~2e-3
                        # rel on the logits - in budget.
                        nc3 = npg * C
                        t8 = sp.tile([128, NPG_MAX * C * 8], BF16, tag="t8")
                        tv = tmp[:, :w].rearrange("p (j d c) -> p j d c", d=D, c=C)
                        t8v = t8[:, : nc3 * 8].rearrange(
                            "p (j d c) -> p j d c", d=8, c=C
                        )
                        nc.vector.tensor_tensor(
                            out=t8v, in0=tv[:, :, 0:8, :], in1=tv[:, :, 8:16, :],
                            op=mybir.AluOpType.add,
                        )
                        t4 = sp.tile([128, NPG_MAX * C * 4], BF16, tag="t4")
                        t4v = t4[:, : nc3 * 4].rearrange(
                            "p (j d c) -> p j d c", d=4, c=C
                        )
                        nc.vector.tensor_tensor(
                            out=t4v, in0=t8v[:, :, 0:4, :], in1=t8v[:, :, 4:8, :],
                            op=mybir.AluOpType.add,
                        )
                        t2 = sp.tile([128, NPG_MAX * C * 2], BF16, tag="t2")
                        t2v = t2[:, : nc3 * 2].rearrange(
                            "p (j d c) -> p j d c", d=2, c=C
                        )
                        nc.vector.tensor_tensor(
                            out=t2v, in0=t4v[:, :, 0:2, :], in1=t4v[:, :, 2:4, :],
                            op=mybir.AluOpType.add,
                        )
                        a = sp.tile([128, NPG_MAX * C], BF16, tag="a")
                        nc.vector.tensor_tensor(
                            out=a[:, :nc3].rearrange("p (j c) -> p j c", c=C
                                                     ).unsqueeze(2),
                            in0=t2v[:, :, 0:1, :], in1=t2v[:, :, 1:2, :],
                            op=mybir.AluOpType.add,
                        )
                        # Pool: b_ij += a
                        bsl = bij[:, p0 * C : (p0 + npg) * C]
                        nc.gpsimd.tensor_tensor(
                            out=bsl, in0=bsl, in1=a[:, :nc3],
                            op=mybir.AluOpType.add,
                        )
                        # ACT: e = exp(b) (unnormalized; 1/E goes into sel)
                        e = sp.tile([128, NPG_MAX * C], BF16, tag="e")
                        nc.scalar.activation(
                            e[:, : npg * C], bsl, mybir.ActivationFunctionType.Exp
                        )
                        # DVE: E = sum_c e ; rs = 1/E ; selv_j = sel * rs_j
                        sm = sp.tile([128, NPG_MAX], F32, tag="sm")
                        nc.vector.tensor_reduce(
                            out=sm[:, :npg],
                            in_=e[:, : npg * C].rearrange("p (j c) -> p j c", c=C),
                            axis=mybir.AxisListType.X,
                            op=mybir.AluOpType.add,
                        )
                        rs = sp.tile([128, NPG_MAX], F32, tag="rs")
                        nc.vector.reciprocal(rs[:, :npg], sm[:, :npg])
                        # ACT: cn = e * 1/E (per-partition scale rides the
                        # activation Copy; selector stationary stays constant)
                        cn = sp.tile([128, NPG_MAX * C], BF16, tag="cn")
                        for j in range(npg):
                            nc.scalar.activation(
                                cn[:, j * C : (j + 1) * C],
                                e[:, j * C : (j + 1) * C],
                                mybir.ActivationFunctionType.Copy,
                                scale=rs[:, j : j + 1],
                            )
                        # y = u * e (2x bf16: e broadcast over the OUTER D
                        # dim, innermost c stays packed)
                        y = wp.tile([128, NPG_MAX * CD], BF16, tag="y")
                        nc.vector.tensor_tensor(
                            out=y[:, :w].rearrange(
                                "p (j d c) -> p j d c", d=D, c=C
                            ),
                            in0=u_sb[:, :w].rearrange(
                                "p (j d c) -> p j d c", d=D, c=C
                            ),
                            in1=cn[:, : npg * C].rearrange("p (j c) -> p j c", c=C
                                                           ).unsqueeze(2)
                                .broadcast_to([128, npg, D, C]),
                            op=mybir.AluOpType.mult,
                        )
                        # PE: s += sel^T @ y
                        for j in range(npg):
                            p = p0 + j
                            nc.tensor.matmul(
                                s_ps[:],
                                lhsT=sel[:],
                                rhs=y[:, j * CD : (j + 1) * CD],
                                start=(p == 0),
                                stop=(p == NPAIR - 1),
                                skip_group_check=True,
                            )

                # all-reduce across the 8 cores via direct remote DMAs:
                # send my partial to peer my_id^k, landing in its slot k.
                # Each broadcast instruction has one live dest at slot k
                # (own DMA lane pair); receiver's rsem gains 2 per arrival.
                nc.scalar.activation(
                    slotsA[0:B, 0:CD], s_ps[:],
                    mybir.ActivationFunctionType.Copy,
                )
                for k in range(1, ncores):
                    rd = [None] * 8
                    rd[k] = (0, k)
                    nc.gpsimd.remote_dma_broadcast(
                        out_ap=slotsA[:, k * CD : (k + 1) * CD],
                        in_ap=slotsA[:, 0:CD],
                        remote_sem=rsem,
                        local_sem=lsem,
                        rdests=rd,
                    )
                s2 = ep.tile([B, CD], F32, tag="s2")
                if ncores > 1:
                    nc.gpsimd.trigger_dma(count=None)
                    # rsem waits are injected post-build (the tile scheduler
                    # would deadlock on a peer-incremented semaphore)
                    sv = slotsA[0:B, :].rearrange("p (s q) -> p s q", q=CD)
                    t4s = ep.tile([B, 4 * CD], F32, tag="t4s")
                    t4sv = t4s[:].rearrange("p (s q) -> p s q", q=CD)
                    nc.vector.tensor_tensor(
                        out=t4sv, in0=sv[:, 0:4, :], in1=sv[:, 4:8, :],
                        op=mybir.AluOpType.add,
                    )
                    t2s = ep.tile([B, 2 * CD], F32, tag="t2s")
                    t2sv = t2s[:].rearrange("p (s q) -> p s q", q=CD)
                    nc.vector.tensor_tensor(
                        out=t2sv, in0=t4sv[:, 0:2, :], in1=t4sv[:, 2:4, :],
                        op=mybir.AluOpType.add,
                    )
                    nc.vector.tensor_tensor(
                        out=s2[:].unsqueeze(1), in0=t2sv[:, 0:1, :],
                        in1=t2sv[:, 1:2, :], op=mybir.AluOpType.add,
                    )
                else:
                    nc.vector.tensor_copy(out=s2[:], in_=slotsA[0:B, 0:CD])

                # squash; round 1's s is unnormalized by 1/C
                v_sb = ep.tile([B, CD], F32, tag="v_sb")
                pre = (1.0 / C) if r == 0 else 1.0
                _squash_chain(nc, ep, sp, s2, v_sb, pre)

                if r < ROUNDS - 1:
                    # broadcast v to both partition halves for next round
                    nc.gpsimd.dma_start(vbc[0:B, :], v_sb[:])
                    nc.gpsimd.dma_start(vbc[B : 2 * B, :], v_sb[:])
                else:
                    nc.sync.dma_start(out_ext, v_sb[:])
    if ncores > 1:
        _inject_allreduce_waits(nc, ncores)
    from concourse.library_overlay import lower_extended_insts
    lower_extended_insts(nc)
    _split_waits(nc)
    return nc


def _inject_allreduce_waits(nc, ncores):
    rsem = nc._allred_rsem
    lsem = nc._allred_lsem
    slots_name = nc._allred_slots_tensor
    per_round = 2 * (ncores - 1)
    lsem_per_round = 16 * (ncores - 1)

    def mkwait(tag, sem, val, engine):
        wait = mybir.SyncWait(
            sync_type="semaphore",
            id=sem.num,
            ant_name=sem.name,
            wait_mode="sem-ge-imm",
            wait_value=val,
            wait_reg=None,
        )
        nop = mybir.InstNoOp(
            name=tag,
            engine=engine,
            sync_info=mybir.SyncInfo(on_wait=[wait], on_update=[]),
        )
        nc.register_instruction(nop, overwrite=True)
        return nop

    def refs(aps, name):
        return any(name in str(ap) for ap in aps)

    rnd = 0
    ev = 0
    for bb_wrap in nc.bb_map.values():
        bb = bb_wrap.bb
        newlist = []
        for inst in bb.instructions:
            # receiver: first sum-add waits for all peer partials
            if (
                isinstance(inst, mybir.InstTensorTensor)
                and inst.engine == mybir.EngineType.DVE
                and refs(inst.ins, slots_name)
            ):
                rnd += 1
                newlist.append(
                    mkwait(f"allred-rwait-{rnd}", rsem, per_round * rnd,
                           inst.engine)
                )
            # sender: round r's evac into slot 0 waits until round r-1's
            # outbound sends have finished reading it
            if (
                isinstance(inst, mybir.InstActivation)
                and refs(inst.outs, slots_name)
            ):
                ev += 1
                if ev > 1:
                    newlist.append(
                        mkwait(f"allred-lwait-{ev}", lsem,
                               lsem_per_round * (ev - 1), inst.engine)
                    )
            newlist.append(inst)
        bb.instructions = newlist
    assert rnd == ROUNDS, f"expected {ROUNDS} sum heads, found {rnd}"
    assert ev == ROUNDS, f"expected {ROUNDS} slot evacs, found {ev}" 


def _prep_core_inputs(x_np, w_np, core):
    """x_np [B, I, d] f32; w_np [I, C, D, d] f32 -> per-core bf16 operands."""
    import ml_dtypes

    lo = core * I_LOC
    xk = x_np[:, lo : lo + I_LOC, :]  # [B, 512, 16]
    wk = w_np[lo : lo + I_LOC]  # [512, C, D, d]

    # W pair tiles: [NPAIR, 32, CD]; rows 0:16 = i0 (d-major), 16:32 = i1
    # free axis in (D, c) order: D-major so downstream broadcasts over D
    # keep the innermost (c) packed
    wt = np.zeros((NPAIR, 32, CD), dtype=np.float32)
    w_dcd = wk.transpose(0, 3, 2, 1).reshape(I_LOC, DSMALL, CD)  # [i, d, (D c)]
    wt[:, 0:DSMALL, :] = w_dcd[0::2]
    wt[:, DSMALL:32, :] = w_dcd[1::2]
    # strip-pack: pair p=4g+s -> partitions [32s,32s+32), free block g
    wsb = wt.reshape(NGRP, 4, 32, CD).transpose(1, 2, 0, 3).reshape(128, NGRP * CD)

    # x block-diag pair tiles: [NPAIR, 32, 128]
    xdg = np.zeros((NPAIR, 32, 128), dtype=np.float32)
    xT = xk.transpose(1, 2, 0)  # [i, d, B]
    xdg[:, 0:DSMALL, 0:B] = xT[0::2]
    xdg[:, DSMALL:32, B : 2 * B] = xT[1::2]
    xsb = xdg.reshape(NGRP, 4, 32, 128).transpose(1, 2, 0, 3).reshape(128, NGRP * 128)

    # round-1 stationary: xr block g rows match wt block g's (i,d) rows,
    # cols = b.  row (32s + 16h + d) <-> x[b, 8g + 2s + h, d]
    xr = np.zeros((NGRP, 128, B), dtype=np.float32)
    for s in range(4):
        for h in range(2):
            rows = slice(32 * s + 16 * h, 32 * s + 16 * h + 16)
            # i_local = 8g + 2s + h for all g: [NGRP]
            idx = 8 * np.arange(NGRP) + 2 * s + h
            # xT[idx] : [NGRP, d, B]
            xr[:, rows, :] = xT[idx]
    xrsb = xr.transpose(1, 0, 2).reshape(128, NGRP * B)

    return {
        "xd": xsb.astype(ml_dtypes.bfloat16),
        "wt": wsb.astype(ml_dtypes.bfloat16),
        "xr": xrsb.astype(ml_dtypes.bfloat16),
    }


_NC_CACHE = {}


def kernel(x: np.ndarray, weights: np.ndarray) -> np.ndarray:
    import ml_dtypes

    x = np.asarray(x, dtype=np.float32)
    w = np.asarray(weights, dtype=np.float32)[0]  # [I, C, D, d]

    if "nc" not in _NC_CACHE:
        _NC_CACHE["nc"] = build_bass()
    nc = _NC_CACHE["nc"]

    selmask = np.zeros((128, B), dtype=np.float32)
    for p in range(128):
        selmask[p, p % B] = 1.0

    in_maps = []
    for core in range(NCORES):
        m = _prep_core_inputs(x, w, core)
        m["sel"] = selmask.astype(ml_dtypes.bfloat16)
        in_maps.append(m)

    res = run_bass_kernel_spmd(nc, in_maps, list(range(NCORES)))
    out = np.asarray(res.results[0]["out"], dtype=np.float32)  # [B, (D c)]
    return out.reshape(B, D, C).transpose(0, 2, 1)


# revision 31
# speedup vs baseline: 16.4957x; 16.4957x over previous
"""DigitCapsules dynamic-routing kernel for 8 TRN2 NeuronCores.

Strategy: shard the input-capsule axis I=4096 across 8 cores (512 each).

Round 1 (uniform c=1/C) collapses to a direct contraction
s1 = (1/C) sum_{i,d} W[(i,d),(c,D)] x[b,(i,d)]: 64 accumulating matmuls
with K=128 (x-chunk stationary, W streaming) - no per-pair work at all.

Rounds 2-3 re-form u_hat pair-by-pair on the TensorEngine (block-diag
K=32 matmuls, 4-way row-tiling), processed in groups of 3 pairs:
  ACT  : evacuate u_hat PSUM->SBUF bf16 (one op/group), exp(b_ij)
  DVE  : tmp = u*v (2x bf16), softmax denom+recip, per-pair selector
         values (1/E folded INTO the selector matmul - no cn pass),
         y = u*e (2x bf16, e pre-expanded over D by an idle DMA queue)
  Pool : segmented reduce sum_D tmp -> agreement a, b_ij += a
  PE   : formation + selector matmuls (s-accumulation in PSUM)
Only the tiny per-round s_j partial [64,512] is all-reduced.

B=64, I=4096, C=32, D=16, d=16, 3 routing iterations.
"""

import numpy as np

import concourse.bass as bass
import concourse.mybir as mybir
from concourse import tile
from concourse.bass_utils import run_bass_kernel_spmd

B = 64
I_FULL = 4096
C = 32
D = 16
DSMALL = 16
CD = C * D  # 512
NCORES = 8
I_LOC = I_FULL // NCORES  # 512
NPAIR = I_LOC // 2  # 256
NGRP = NPAIR // 4  # 64 groups of 4 pairs (one pair per 32-row strip)
ROUNDS = 3
EPS = 1e-9

F32 = mybir.dt.float32
BF16 = mybir.dt.bfloat16


def _split_waits(nc, max_waits=1):
    """walrus in this toolchain rejects instructions carrying more than
    ~2 semaphore waits; move extras onto preceding same-engine NOPs."""
    for bb_wrap in nc.bb_map.values():
        bb = bb_wrap.bb
        newlist = []
        changed = False
        for inst in bb.instructions:
            si = inst.sync_info
            waits = list(si.on_wait) if si and si.on_wait else []
            if len(waits) > max_waits:
                extra, keep = waits[:-max_waits], waits[-max_waits:]
                k = 0
                while extra:
                    chunk, extra = extra[:max_waits], extra[max_waits:]
                    nop = mybir.InstNoOp(
                        name=f"{inst.name}-waitsplit{k}",
                        engine=inst.engine,
                        sync_info=mybir.SyncInfo(on_wait=chunk, on_update=[]),
                    )
                    nc.register_instruction(nop, overwrite=True)
                    newlist.append(nop)
                    k += 1
                inst.sync_info = mybir.SyncInfo(
                    on_wait=keep,
                    on_update=list(si.on_update) if si.on_update else [],
                )
                changed = True
            newlist.append(inst)
        if changed:
            bb.instructions = newlist
    return nc


# pair-group partition for the heavy rounds: super-groups of 6 pairs.
# One 6-bank PSUM tile per group; formation of g+1 waits on g's single
# ACT evacuation (PE has slack).  6 uh + 1 s1 + 1 s-accum = 8 banks.
GROUPS = [(6 * k, 6) for k in range(42)] + [(252, 4)]
assert sum(n for _, n in GROUPS) == NPAIR
NPG_MAX = 6


def _squash_chain(nc, ep, sp, s2, v_sb, pre_scale):
    """v = squash(pre_scale * s2), on [B, CD] fp32 tiles in (D, c) free
    order.  squash(s): s / (1+n2) / sqrt(n2+eps), n2 = sum_D s^2.
    Folding: with t = k*s (k=pre_scale), n2 = k^2 * sum(s^2);
    v = t * scl with scl = 1/((1+n2)*sqrt(n2+eps)) computed on n2."""
    sq = ep.tile([B, CD], F32, tag="sq")
    nc.vector.tensor_tensor(out=sq[:], in0=s2[:], in1=s2[:], op=mybir.AluOpType.mult)
    q = sp.tile([B, C], F32, tag="q")
    nc.vector.tensor_reduce(
        out=q[:],
        in_=sq[:].rearrange("p (d c) -> p c d", d=D),
        axis=mybir.AxisListType.X,
        op=mybir.AluOpType.add,
    )
    k2 = pre_scale * pre_scale
    n2 = sp.tile([B, C], F32, tag="n2")
    nc.vector.tensor_scalar_mul(n2[:], q[:], k2)
    n2e = sp.tile([B, C], F32, tag="n2e")
    nc.vector.tensor_scalar_add(n2e[:], n2[:], EPS)
    rt = sp.tile([B, C], F32, tag="rt")
    nc.scalar.activation(rt[:], n2e[:], mybir.ActivationFunctionType.Sqrt)
    on2 = sp.tile([B, C], F32, tag="on2")
    nc.vector.tensor_scalar_add(on2[:], n2[:], 1.0)
    den = sp.tile([B, C], F32, tag="den")
    nc.vector.tensor_tensor(
        out=den[:], in0=rt[:], in1=on2[:], op=mybir.AluOpType.mult
    )
    scl = sp.tile([B, C], F32, tag="scl")
    nc.vector.reciprocal(scl[:], den[:])
    # v = (k*s) * scl : fold k into scl first (cheap [B,C] op)
    sclk = sp.tile([B, C], F32, tag="sclk")
    nc.vector.tensor_scalar_mul(sclk[:], scl[:], pre_scale)
    nc.vector.tensor_tensor(
        out=v_sb[:].rearrange("p (d c) -> p d c", c=C),
        in0=s2[:].rearrange("p (d c) -> p d c", c=C),
        in1=sclk[:].unsqueeze(1).broadcast_to([B, D, C]),
        op=mybir.AluOpType.mult,
    )


def build_bass(ncores=NCORES):
    nc = bass.Bass(
        "TRN2", target_bir_lowering=False, debug=False, num_devices=ncores
    )
    xd_ext = nc.dram_tensor("xd", [128, NGRP * 128], BF16, kind="ExternalInput").ap()
    wt_ext = nc.dram_tensor("wt", [128, NGRP * CD], BF16, kind="ExternalInput").ap()
    xr_ext = nc.dram_tensor("xr", [128, NGRP * B], BF16, kind="ExternalInput").ap()
    sel_ext = nc.dram_tensor("sel", [128, B], BF16, kind="ExternalInput").ap()
    out_ext = nc.dram_tensor("out", [B, CD], F32, kind="ExternalOutput").ap()

    with tile.TileContext(nc) as tc:
        with (
            tc.tile_pool(name="persist", bufs=1) as pp,
            tc.tile_pool(name="work", bufs=2) as wp,
            tc.tile_pool(name="epi", bufs=2) as ep,
            tc.tile_pool(name="usb", bufs=6) as up,
            tc.tile_pool(name="small", bufs=2) as sp,
            tc.tile_pool(name="uh", bufs=2, space="PSUM") as uhp,
            tc.tile_pool(name="sacc1", bufs=1, space="PSUM") as sacc1p,
            tc.tile_pool(name="sacc", bufs=1, space="PSUM") as saccp,
            tc.tile_pool(name="dram", bufs=2, space="DRAM") as dp,
        ):
            xd = pp.tile([128, NGRP * 128], BF16)
            wt = pp.tile([128, NGRP * CD], BF16)
            xr = pp.tile([128, NGRP * B], BF16)
            sel = pp.tile([128, B], BF16)
            bij = pp.tile([128, NPAIR * C], BF16)
            vbc = pp.tile([128, CD], BF16)

            # stage inputs; wt in 4 chunks so round-1 matmuls can start
            # before the whole tensor has landed
            nc.sync.dma_start(xr[:], xr_ext)
            nc.sync.dma_start(sel[:], sel_ext)
            WCH = NGRP // 4
            for ch in range(4):
                nc.sync.dma_start(
                    wt[:, ch * WCH * CD : (ch + 1) * WCH * CD],
                    wt_ext[:, ch * WCH * CD : (ch + 1) * WCH * CD],
                )
            nc.sync.dma_start(xd[:], xd_ext)
            nc.vector.memset(bij[:], 0.0)

            # ---------------- round 1: direct contraction --------------
            # s1_raw[b, cD] = sum_g  xr_g[128,(b)]^T @ wt[:, g-block]
            # (wt block g's 128 partitions are exactly (i,d) for the 8
            # capsules of that block; xr holds x in the matching layout)
            s1_ps = sacc1p.tile([B, CD], F32)
            for g in range(NGRP):
                nc.tensor.matmul(
                    s1_ps[:],
                    lhsT=xr[:, g * B : (g + 1) * B],
                    rhs=wt[:, g * CD : (g + 1) * CD],
                    start=(g == 0),
                    stop=(g == NGRP - 1),
                    skip_group_check=True,
                )

            for r in range(ROUNDS):
                if r == 0:
                    s_ps = s1_ps
                else:
                    s_ps = saccp.tile([B, CD], F32)

                    # producer: form a group on PE, evacuate to SBUF bf16 on
                    # ACT.  Runs LAG groups ahead of the consumer so PE/ACT
                    # fill buffers during collectives and round starts.
                    usb_tiles = {}

                    def form_group(gi):
                        p0, npg = GROUPS[gi]
                        u_sb = up.tile([128, NPG_MAX * CD], BF16, tag="usb")
                        # two 3-bank halves: formation of one half overlaps
                        # the ACT evacuation of the other
                        for h0 in range(0, npg, 3):
                            nh = min(3, npg - h0)
                            uh = uhp.tile([128, 3 * CD], F32, tag="uh")
                            for j in range(nh):
                                p = p0 + h0 + j
                                g, st = p // 4, p % 4
                                nc.tensor.matmul(
                                    uh[:, j * CD : (j + 1) * CD],
                                    lhsT=xd[32 * st : 32 * st + 32, g * 128 : (g + 1) * 128],
                                    rhs=wt[32 * st : 32 * st + 32, g * CD : (g + 1) * CD],
                                    start=True,
                                    stop=True,
                                    tile_position=(32 * st, 0),
                                )
                            nc.scalar.activation(
                                u_sb[:, (h0) * CD : (h0 + nh) * CD],
                                uh[:, : nh * CD],
                                mybir.ActivationFunctionType.Copy,
                            )
                        usb_tiles[gi] = u_sb

                    LAG = 4
                    for gi in range(LAG):
                        form_group(gi)
                    for gi, (p0, npg) in enumerate(GROUPS):
                        if gi + LAG < len(GROUPS):
                            form_group(gi + LAG)
                        w = npg * CD
                        u_sb = usb_tiles.pop(gi)
                        # DVE: tmp = u * v (2x bf16; v broadcast per pair
                        # via middle-dim 0-stride, innermost stays packed)
                        # u free layout is (pair, D, c): D-major so later
                        # broadcasts/reductions keep the innermost packed.
                        tmp = wp.tile([128, NPG_MAX * CD], BF16, tag="tmp")
                        nc.vector.tensor_tensor(
                            out=tmp[:, :w].rearrange("p (j q) -> p j q", q=CD),
                            in0=u_sb[:, :w].rearrange("p (j q) -> p j q", q=CD),
                            in1=vbc[:].unsqueeze(1).broadcast_to([128, npg, CD]),
                            op=mybir.AluOpType.mult,
                        )
                        # DVE: agreement a = sum_D tmp as a 4-step halving
                        # tree over the OUTER D dim (TT-adds run 2x on
                        # packed bf16; TensorReduce would be stuck at 1x).
                        # |a|~0.04 on |b|~0.1: bf16 partials cost ~2e-3
                        # rel on the logits - in budget.
                        nc3 = npg * C
                        t8 = sp.tile([128, NPG_MAX * C * 8], BF16, tag="t8")
                        tv = tmp[:, :w].rearrange("p (j d c) -> p j d c", d=D, c=C)
                        t8v = t8[:, : nc3 * 8].rearrange(
                            "p (j d c) -> p j d c", d=8, c=C
                        )
                        nc.vector.tensor_tensor(
                            out=t8v, in0=tv[:, :, 0:8, :], in1=tv[:, :, 8:16, :],
                            op=mybir.AluOpType.add,
                        )
                        t4 = sp.tile([128, NPG_MAX * C * 4], BF16, tag="t4")
                        t4v = t4[:, : nc3 * 4].rearrange(
                            "p (j d c) -> p j d c", d=4, c=C
                        )
                        nc.vector.tensor_tensor(
                            out=t4v, in0=t8v[:, :, 0:4, :], in1=t8v[:, :, 4:8, :],
                            op=mybir.AluOpType.add,
                        )
                        t2 = sp.tile([128, NPG_MAX * C * 2], BF16, tag="t2")
                        t2v = t2[:, : nc3 * 2].rearrange(
                            "p (j d c) -> p j d c", d=2, c=C
                        )
                        nc.vector.tensor_tensor(
                            out=t2v, in0=t4v[:, :, 0:2, :], in1=t4v[:, :, 2:4, :],
                            op=mybir.AluOpType.add,
                        )
                        a = sp.tile([128, NPG_MAX * C], BF16, tag="a")
                        nc.vector.tensor_tensor(
                            out=a[:, :nc3].rearrange("p (j c) -> p j c", c=C
                                                     ).unsqueeze(2),
                            in0=t2v[:, :, 0:1, :], in1=t2v[:, :, 1:2, :],
                            op=mybir.AluOpType.add,
                        )
                        # Pool: b_ij += a
                        bsl = bij[:, p0 * C : (p0 + npg) * C]
                        nc.gpsimd.tensor_tensor(
                            out=bsl, in0=bsl, in1=a[:, :nc3],
                            op=mybir.AluOpType.add,
                        )
                        # ACT: e = exp(b) (unnormalized; 1/E goes into sel)
                        e = sp.tile([128, NPG_MAX * C], BF16, tag="e")
                        nc.scalar.activation(
                            e[:, : npg * C], bsl, mybir.ActivationFunctionType.Exp
                        )
                        # DVE: E = sum_c e ; rs = 1/E ; selv_j = sel * rs_j
                        sm = sp.tile([128, NPG_MAX], F32, tag="sm")
                        nc.vector.tensor_reduce(
                            out=sm[:, :npg],
                            in_=e[:, : npg * C].rearrange("p (j c) -> p j c", c=C),
                            axis=mybir.AxisListType.X,
                            op=mybir.AluOpType.add,
                        )
                        rs = sp.tile([128, NPG_MAX], F32, tag="rs")
                        nc.vector.reciprocal(rs[:, :npg], sm[:, :npg])
                        # ACT: cn = e * 1/E (per-partition scale rides the
                        # activation Copy; selector stationary stays constant)
                        cn = sp.tile([128, NPG_MAX * C], BF16, tag="cn")
                        for j in range(npg):
                            nc.scalar.activation(
                                cn[:, j * C : (j + 1) * C],
                                e[:, j * C : (j + 1) * C],
                                mybir.ActivationFunctionType.Copy,
                                scale=rs[:, j : j + 1],
                            )
                        # y = u * e (2x bf16: e broadcast over the OUTER D
                        # dim, innermost c stays packed)
                        y = wp.tile([128, NPG_MAX * CD], BF16, tag="y")
                        nc.vector.tensor_tensor(
                            out=y[:, :w].rearrange(
                                "p (j d c) -> p j d c", d=D, c=C
                            ),
                            in0=u_sb[:, :w].rearrange(
                                "p (j d c) -> p j d c", d=D, c=C
                            ),
                            in1=cn[:, : npg * C].rearrange("p (j c) -> p j c", c=C
                                                           ).unsqueeze(2)
                                .broadcast_to([128, npg, D, C]),
                            op=mybir.AluOpType.mult,
                        )
                        # PE: s += sel^T @ y
                        for j in range(npg):
                            p = p0 + j
                            nc.tensor.matmul(
                                s_ps[:],
                                lhsT=sel[:],
                                rhs=y[:, j * CD : (j + 1) * CD],
                                start=(p == 0),
                                stop=(p == NPAIR - 1),
                                skip_group_check=True,
                            )

                # evacuate s partial, all-reduce across the 8 cores
                s_sb = ep.tile([B, CD], F32, tag="s_sb")
                nc.scalar.activation(
                    s_sb[:], s_ps[:], mybir.ActivationFunctionType.Copy
                )
                ccin = dp.tile([B, CD], F32, tag="ccin")
                ccout = dp.tile([B, CD], F32, tag="ccout")
                nc.gpsimd.dma_start(ccin[:], s_sb[:])
                nc.gpsimd.collective_compute(
                    "AllReduce",
                    mybir.AluOpType.add,
                    replica_groups=[list(range(ncores))],
                    ins=[ccin[:].opt()],
                    outs=[ccout[:].opt()],
                )
                s2 = ep.tile([B, CD], F32, tag="s2")
                nc.gpsimd.dma_start(s2[:], ccout[:])

                # squash; round 1's s is unnormalized by 1/C
                v_sb = ep.tile([B, CD], F32, tag="v_sb")
                pre = (1.0 / C) if r == 0 else 1.0
                _squash_chain(nc, ep, sp, s2, v_sb, pre)

                if r < ROUNDS - 1:
                    # broadcast v to both partition halves for next round
                    nc.gpsimd.dma_start(vbc[0:B, :], v_sb[:])
                    nc.gpsimd.dma_start(vbc[B : 2 * B, :], v_sb[:])
                else:
                    nc.sync.dma_start(out_ext, v_sb[:])
    _split_waits(nc)
    return nc


def _prep_core_inputs(x_np, w_np, core):
    """x_np [B, I, d] f32; w_np [I, C, D, d] f32 -> per-core bf16 operands."""
    import ml_dtypes

    lo = core * I_LOC
    xk = x_np[:, lo : lo + I_LOC, :]  # [B, 512, 16]
    wk = w_np[lo : lo + I_LOC]  # [512, C, D, d]

    # W pair tiles: [NPAIR, 32, CD]; rows 0:16 = i0 (d-major), 16:32 = i1
    # free axis in (D, c) order: D-major so downstream broadcasts over D
    # keep the innermost (c) packed
    wt = np.zeros((NPAIR, 32, CD), dtype=np.float32)
    w_dcd = wk.transpose(0, 3, 2, 1).reshape(I_LOC, DSMALL, CD)  # [i, d, (D c)]
    wt[:, 0:DSMALL, :] = w_dcd[0::2]
    wt[:, DSMALL:32, :] = w_dcd[1::2]
    # strip-pack: pair p=4g+s -> partitions [32s,32s+32), free block g
    wsb = wt.reshape(NGRP, 4, 32, CD).transpose(1, 2, 0, 3).reshape(128, NGRP * CD)

    # x block-diag pair tiles: [NPAIR, 32, 128]
    xdg = np.zeros((NPAIR, 32, 128), dtype=np.float32)
    xT = xk.transpose(1, 2, 0)  # [i, d, B]
    xdg[:, 0:DSMALL, 0:B] = xT[0::2]
    xdg[:, DSMALL:32, B : 2 * B] = xT[1::2]
    xsb = xdg.reshape(NGRP, 4, 32, 128).transpose(1, 2, 0, 3).reshape(128, NGRP * 128)

    # round-1 stationary: xr block g rows match wt block g's (i,d) rows,
    # cols = b.  row (32s + 16h + d) <-> x[b, 8g + 2s + h, d]
    xr = np.zeros((NGRP, 128, B), dtype=np.float32)
    for s in range(4):
        for h in range(2):
            rows = slice(32 * s + 16 * h, 32 * s + 16 * h + 16)
            # i_local = 8g + 2s + h for all g: [NGRP]
            idx = 8 * np.arange(NGRP) + 2 * s + h
            # xT[idx] : [NGRP, d, B]
            xr[:, rows, :] = xT[idx]
    xrsb = xr.transpose(1, 0, 2).reshape(128, NGRP * B)

    return {
        "xd": xsb.astype(ml_dtypes.bfloat16),
        "wt": wsb.astype(ml_dtypes.bfloat16),
        "xr": xrsb.astype(ml_dtypes.bfloat16),
    }


_NC_CACHE = {}


def kernel(x: np.ndarray, weights: np.ndarray) -> np.ndarray:
    import ml_dtypes

    x = np.asarray(x, dtype=np.float32)
    w = np.asarray(weights, dtype=np.float32)[0]  # [I, C, D, d]

    if "nc" not in _NC_CACHE:
        _NC_CACHE["nc"] = build_bass()
    nc = _NC_CACHE["nc"]

    selmask = np.zeros((128, B), dtype=np.float32)
    for p in range(128):
        selmask[p, p % B] = 1.0

    in_maps = []
    for core in range(NCORES):
        m = _prep_core_inputs(x, w, core)
        m["sel"] = selmask.astype(ml_dtypes.bfloat16)
        in_maps.append(m)

    res = run_bass_kernel_spmd(nc, in_maps, list(range(NCORES)))
    out = np.asarray(res.results[0]["out"], dtype=np.float32)  # [B, (D c)]
    return out.reshape(B, D, C).transpose(0, 2, 1)
